# revision 19
# baseline (speedup 1.0000x reference)
"""Trainium2 Bass kernel for the Bengio03 Highway BiLM problem.

Math (see reference): L=3 layers, each with fwd/bwd chains. Per (layer, dir):
  padded = [front_pads(4), seq(512), back_pads(4)]          # [B, 520, H]
  pre[t] = sum_{k=0..4} padded[t + k + off] @ W[k*H:(k+1)*H]  (off=0 fwd, 4 bwd)
  x0 = relu(pre + b)
  2x highway: proj = x @ Ws[j] + bs[j]; nonlin,gate = split(proj)
              x = sigmoid(gate)*x + (1-sigmoid(gate))*relu(nonlin)
  out[l,:,:, 0:512] = f chain, [512:1024] = b chain

Implementation strategy (per core, data-parallel over batch: 4 seqs/core):
  - Everything in bf16 (fp32 PSUM accumulation): same PE rate as fp32r but
    2-byte weight loads (no LDWEIGHTS bubble), 2x DVE rate, half the DMA.
  - Activations kept feature-major in SBUF: xT tiles [128(h), 520(t)] per
    (h_chunk, b). Matmuls compute outT = W_tile.T @ xT directly (weights are
    the stationary lhsT, activations stream as rhs with N=512 tokens), so no
    transposes are needed between layers. The 5-tap conv is 5 shifted rhs
    slices accumulated in PSUM (20 matmuls of [128,128]@[128,512] per output
    chunk).
  - Input arrives pre-transposed (feature-major) from the host; output is
    stored feature-major and re-transposed on the host. The device does no
    layout work at all: only matmuls, elementwise, and linear DMAs.
  - DMA trigger instructions cost ~650ns each on the dispatching engine, so
    weights are host-packed into big blocks: 4 DMAs per conv stage, 2 per
    highway stage, 5 for all constants. Triggers round-robin sync/scalar/
    gpsimd queues (~350 GB/s aggregate).
  - Elementwise: ScalarE does relu/sigmoid (+bias, PSUM->SBUF), VectorE does
    the 3 tensor_tensor ops of the highway combine, GpSimd writes pad columns.
"""

import os
import sys

sys.path.insert(0, "/opt/trn_rl_repo")

import ml_dtypes
import numpy as np

import concourse.bass as bass
import concourse.bacc as bacc
import concourse.tile as tile
from concourse import mybir
from concourse.bass_utils import run_bass_kernel_spmd

# Problem constants (hardcoded per spec).
L = 3
WIDTH = 4
H = 512
B = 32
S = 512
NHW = 2
CIN = (WIDTH + 1) * H  # 2560
NCORES = 8
BLOC = B // NCORES  # 4 sequences per core
HC = H // 128  # 4 hidden chunks of 128
SPAD = S + 2 * WIDTH  # 520
NSEC = 4  # conv weight sections per stage (5 k-chunks each)
SECR = CIN // 128 // NSEC  # 5
F32 = mybir.dt.float32
BF16 = mybir.dt.bfloat16
RELU = mybir.ActivationFunctionType.Relu
SIGM = mybir.ActivationFunctionType.Sigmoid
ADD = mybir.AluOpType.add
MAX = mybir.AluOpType.max

NPBF16 = ml_dtypes.bfloat16


def _build_program():
    nc = bacc.Bacc(
        "TRN2",
        target_bir_lowering=False,
        debug=False,
        enable_asserts=False,
        num_devices=1,
    )

    # Input pre-transposed to feature-major on host: [b, h_chunk, h_in_chunk, t]
    xT_d = nc.dram_tensor("xT", [BLOC, HC, 128, S], BF16, kind="ExternalInput").ap()
    # Conv weights: [l, r-chunk, part, H] (per-tile DMAs)
    fw_d = nc.dram_tensor("fw", [L, CIN // 128, 128, H], BF16, kind="ExternalInput").ap()
    bw_d = nc.dram_tensor("bw", [L, CIN // 128, 128, H], BF16, kind="ExternalInput").ap()
    # Highway weights: [l, j, h_chunk, part, 2H] (per-tile DMAs)
    fhw_d = nc.dram_tensor("fhw", [L, NHW, HC, 128, 2 * H], BF16, kind="ExternalInput").ap()
    bhw_d = nc.dram_tensor("bhw", [L, NHW, HC, 128, 2 * H], BF16, kind="ExternalInput").ap()
    # Biases host-packed into single planes: [128, L*HC] / [128, L*NHW*2HC]
    cb_d = nc.dram_tensor("cb", [2, 128, L * HC], F32, kind="ExternalInput").ap()
    hb_d = nc.dram_tensor("hb", [2, 128, L * NHW * 2 * HC], F32, kind="ExternalInput").ap()
    # Pads host-packed: [2, 128, L*HC*WIDTH]
    pad_d = nc.dram_tensor("pad", [2, 128, L * HC * WIDTH], BF16, kind="ExternalInput").ap()
    # Output feature-major: [l, b, feat_chunk(2H/128=8), 128, t]; host transposes.
    out_d = nc.dram_tensor("out", [L, BLOC, 2 * HC, 128, S], BF16, kind="ExternalOutput").ap()
    # Startup fence target: a dependent dummy store per dispatch engine keeps
    # second-wave weight DMAs from competing with the critical first loads.
    fence_d = nc.dram_tensor("fence", [3, 128, 1], BF16, kind="ExternalOutput").ap()

    with tile.TileContext(nc) as tc:
        with (
            tc.tile_pool(name="consts", bufs=1) as consts,
            tc.tile_pool(name="acts", bufs=48) as acts,
            tc.tile_pool(name="convw", bufs=40) as convw,
            tc.tile_pool(name="hww", bufs=16) as hww,
            tc.tile_pool(name="xmid", bufs=8) as xmid,
            tc.tile_pool(name="work", bufs=4) as work,
            tc.tile_pool(name="psum", bufs=2, space="PSUM") as psum,
        ):
            # ---- constants: 5 small DMAs total ----
            padt = consts.tile([128, 2, L * HC * WIDTH], BF16, name="padt", tag="padt", bufs=1)
            nc.sync.dma_start(padt[:, 0], pad_d[0])
            nc.sync.dma_start(padt[:, 1], pad_d[1])
            cbt = consts.tile([128, 2, L * HC], F32, name="cbt", tag="cbt", bufs=1)
            nc.sync.dma_start(cbt[:, 0], cb_d[0])
            nc.sync.dma_start(cbt[:, 1], cb_d[1])
            hbt = consts.tile([128, 2, L * NHW * 2 * HC], F32, name="hbt", tag="hbt", bufs=1)
            nc.sync.dma_start(hbt[:, 0], hb_d[0])
            nc.sync.dma_start(hbt[:, 1], hb_d[1])

            def cbias(dirc, l, n):
                return cbt[:, 0 if dirc == "f" else 1, l * HC + n: l * HC + n + 1]

            def hbias(dirc, l, j, c):
                o = (l * NHW + j) * 2 * HC + c
                return hbt[:, 0 if dirc == "f" else 1, o:o + 1]

            def pad_ap(dirc, l, c):
                o = (l * HC + c) * WIDTH
                return padt[:, 0 if dirc == "f" else 1, o:o + WIDTH]

            def new_act_tile(name):
                return acts.tile([128, SPAD], BF16, name=name, tag="acts", bufs=48)

            def write_pads(at, l, c):
                nc.gpsimd.tensor_copy(at[:, 0:WIDTH], pad_ap("f", l, c))
                nc.gpsimd.tensor_copy(at[:, WIDTH + S:SPAD], pad_ap("b", l, c))

            QS = (nc.sync, nc.scalar, nc.gpsimd)
            qctr = [0]

            def qnext():
                q = QS[qctr[0] % 3]
                qctr[0] += 1
                return q

            loaded_cw = {}
            loaded_hw = {}

            def ensure_convw(dirc, l):
                if (dirc, l) not in loaded_cw:
                    src = fw_d if dirc == "f" else bw_d
                    tiles = []
                    for r in range(CIN // 128):
                        w = convw.tile(
                            [128, H], BF16, name=f"cw_{dirc}{l}_{r}",
                            tag="convw", bufs=40,
                        )
                        qnext().dma_start(w[:], src[l, r])
                        tiles.append(w)
                    loaded_cw[(dirc, l)] = tiles
                return loaded_cw[(dirc, l)]

            def load_convw_halves(dirc, l):
                # Column-wave DMAs so the first conv groups (low n) can start
                # after only a fraction of the weight bytes have landed.
                src = fw_d if dirc == "f" else bw_d
                tiles = []
                for r in range(CIN // 128):
                    w = convw.tile(
                        [128, H], BF16, name=f"cw_{dirc}{l}_{r}",
                        tag="convw", bufs=40,
                    )
                    tiles.append(w)
                for c0, c1 in ((0, 128), (128, 256), (256, 512)):
                    for r in range(CIN // 128):
                        qnext().dma_start(tiles[r][:, c0:c1], src[l, r][:, c0:c1])
                loaded_cw[(dirc, l)] = tiles
                return tiles

            def ensure_hww(dirc, l):
                if (dirc, l) not in loaded_hw:
                    src = fhw_d if dirc == "f" else bhw_d
                    jt = []
                    for j in range(NHW):
                        ht = []
                        for h in range(HC):
                            w = hww.tile(
                                [128, 2 * H], BF16, name=f"hw_{dirc}{l}_{j}_{h}",
                                tag="hww", bufs=16,
                            )
                            qnext().dma_start(w[:], src[l, j, h])
                            ht.append(w)
                        jt.append(ht)
                    loaded_hw[(dirc, l)] = jt
                return loaded_hw[(dirc, l)]

            # ---- input stage: direct feature-major bf16 load ----
            # Pair (0,1) first (feeds the very first conv groups), then the
            # l0 conv weights, then pair (2,3); all round-robined over queues.
            xT = {}

            def load_x(b):
                for c in range(HC):
                    at = new_act_tile(f"xT_{c}_{b}")
                    qnext().dma_start(at[:, WIDTH:WIDTH + S], xT_d[b, c])
                    write_pads(at, 0, c)
                    xT[(c, b)] = at

            load_x(0)
            load_x(1)
            load_convw_halves("f", 0)

            # ---- stages ----
            def conv_stage(dirc, l, srcset, pair, wtiles, n_outer=False):
                off0 = 0 if dirc == "f" else WIDTH
                x0 = {}
                if n_outer:  # startup: n=0,1 groups only need the first column wave
                    groups = [(b, n) for n in range(HC) for b in pair]
                else:
                    groups = [(b, n) for b in pair for n in range(HC)]
                for b, n in groups:
                        ps = psum.tile([128, S], F32, name=f"cps_{b}_{n}", tag="cpsum", bufs=4)
                        for r in range(20):
                            k, ci = divmod(r, HC)
                            off = off0 + k
                            nc.tensor.matmul(
                                ps[:],
                                lhsT=wtiles[r][:, n * 128:(n + 1) * 128],
                                rhs=srcset[(ci, b)][:, off:off + S],
                                start=(r == 0),
                                stop=(r == 19),
                            )
                        xt = xmid.tile([128, S], BF16, name=f"x0_{b}_{n}", tag="x0", bufs=8)
                        nc.vector.tensor_scalar(
                            xt[:], ps[:], cbias(dirc, l, n), 0.0, ADD, MAX
                        )
                        x0[(n, b)] = xt
                return x0

            def hw_stage(dirc, l, j, srcset, pair, wt, final):
                outs = {}
                for b in pair:
                    for c in range(HC):
                        pnl = psum.tile([128, S], F32, name=f"hnl_{b}_{c}", tag="hpsum", bufs=4)
                        for h in range(HC):
                            nc.tensor.matmul(
                                pnl[:],
                                lhsT=wt[h][:, c * 128:(c + 1) * 128],
                                rhs=srcset[(h, b)][:],
                                start=(h == 0),
                                stop=(h == HC - 1),
                            )
                        pgt = psum.tile([128, S], F32, name=f"hgt_{b}_{c}", tag="hpsum", bufs=4)
                        for h in range(HC):
                            nc.tensor.matmul(
                                pgt[:],
                                lhsT=wt[h][:, H + c * 128:H + (c + 1) * 128],
                                rhs=srcset[(h, b)][:],
                                start=(h == 0),
                                stop=(h == HC - 1),
                            )
                        r = work.tile([128, S], BF16, name=f"r_{b}_{c}", tag="r", bufs=4)
                        nc.scalar.activation(r[:], pnl[:], RELU, bias=hbias(dirc, l, j, c))
                        g = work.tile([128, S], BF16, name=f"g_{b}_{c}", tag="g", bufs=4)
                        nc.scalar.activation(g[:], pgt[:], SIGM, bias=hbias(dirc, l, j, HC + c))
                        d = work.tile([128, S], BF16, name=f"d_{b}_{c}", tag="d", bufs=4)
                        nc.vector.tensor_sub(d[:], srcset[(c, b)][:], r[:])
                        nc.vector.tensor_mul(d[:], g[:], d[:])
                        if final:
                            at = new_act_tile(f"a_{dirc}{l}_{c}_{b}")
                            nc.vector.tensor_add(at[:, WIDTH:WIDTH + S], d[:], r[:])
                            if l + 1 < L:
                                write_pads(at, l + 1, c)
                            emit_out(dirc, l, at, c, b)
                            outs[(c, b)] = at
                        else:
                            o = xmid.tile([128, S], BF16, name=f"x1_{b}_{c}", tag="x1", bufs=8)
                            nc.vector.tensor_add(o[:], d[:], r[:])
                            outs[(c, b)] = o
                return outs

            def emit_out(dirc, l, at, c, b):
                # Feature-major store: one linear DMA per tile; the host undoes
                # the layout. Alternate dispatch queues to spread DMA load.
                # Final stage splits each store across two queues so the last
                # tiles drain faster after the final combine.
                k = (0 if dirc == "f" else HC) + c
                if dirc == "b" and l == L - 1:
                    e0 = QS[(c + b) % 3]
                    e1 = QS[(c + b + 1) % 3]
                    e0.dma_start(out_d[l, b, k][:, 0:S // 2], at[:, WIDTH:WIDTH + S // 2])
                    e1.dma_start(out_d[l, b, k][:, S // 2:], at[:, WIDTH + S // 2:WIDTH + S])
                else:
                    eng = QS[(c + b) % 3]
                    eng.dma_start(out_d[l, b, k], at[:, WIDTH:WIDTH + S])

            # ---- main chain: f fully, then b (xT stays resident for b) ----
            PAIRS = [(0, 1), (2, 3)]
            first = True
            for dirc in ("f", "b"):
                cur = xT
                for l in range(L):
                    cw = ensure_convw(dirc, l)
                    nxt = {}
                    hw = None
                    for pair in PAIRS:
                        x0 = conv_stage(dirc, l, cur, pair, cw, n_outer=first)
                        if first and pair == PAIRS[0]:
                            # Fence: each dispatch engine stalls here until the
                            # first conv group completes, so everything below
                            # streams only after the critical loads finish.
                            for qi in range(3):
                                QS[qi].dma_start(fence_d[qi], x0[(0, pair[0])][:, 0:1])
                            load_x(2)
                            load_x(3)
                        # hww DMAs emitted after the first conv groups so they
                        # don't compete with the critical startup loads.
                        if hw is None:
                            hw = ensure_hww(dirc, l)
                        x1 = hw_stage(dirc, l, 0, x0, pair, hw[0], final=False)
                        res = hw_stage(dirc, l, 1, x1, pair, hw[1], final=True)
                        nxt.update(res)
                    first = False
                    cur = nxt

    nc.compile()
    return nc


_CACHE = {}


def _get_program():
    if "nc" not in _CACHE:
        _CACHE["nc"] = _build_program()
    return _CACHE["nc"]


def _bf16(a):
    return np.ascontiguousarray(np.asarray(a, dtype=np.float32).astype(NPBF16))


def _make_in_maps(inputs):
    x = np.asarray(inputs["inputs"], dtype=np.float32).astype(NPBF16)
    # Conv weights [L, CIN, H] -> [L, 20, 128, H]
    def packw(w):
        w = np.asarray(w, dtype=np.float32).astype(NPBF16)
        return np.ascontiguousarray(w.reshape(L, CIN // 128, 128, H))

    # Highway weights [L, NHW, H, 2H] -> [L, NHW, HC, 128, 2H]
    def packhw(w):
        w = np.asarray(w, dtype=np.float32).astype(NPBF16)
        return np.ascontiguousarray(w.reshape(L, NHW, HC, 128, 2 * H))

    fw = packw(inputs["fwd_W"])
    bw = packw(inputs["bwd_W"])
    fhw = packhw(inputs["fwd_hw_W"])
    bhw = packhw(inputs["bwd_hw_W"])

    # Conv biases [L, H] -> [128, L*HC] stacked f/b
    def packcb(b):
        b = np.asarray(b, dtype=np.float32).reshape(L, HC, 128).transpose(2, 0, 1)
        return b.reshape(128, L * HC)

    cb = np.ascontiguousarray(np.stack([packcb(inputs["fwd_b"]), packcb(inputs["bwd_b"])]))

    # Highway biases [L, NHW, 2H] -> [128, L*NHW*2HC] stacked f/b
    def packhb(b):
        b = np.asarray(b, dtype=np.float32).reshape(L, NHW, 2 * HC, 128).transpose(3, 0, 1, 2)
        return b.reshape(128, L * NHW * 2 * HC)

    hb = np.ascontiguousarray(np.stack([packhb(inputs["fwd_hw_b"]), packhb(inputs["bwd_hw_b"])]))

    # Pads [L, W, H] -> [128, L*HC*W] stacked f/b
    def packpad(p):
        p = np.asarray(p, dtype=np.float32).reshape(L, WIDTH, HC, 128).transpose(3, 0, 2, 1)
        return p.reshape(128, L * HC * WIDTH).astype(NPBF16)

    pad = np.ascontiguousarray(np.stack([packpad(inputs["fwd_pads"]), packpad(inputs["bwd_pads"])]))

    shared = {
        "fw": fw, "bw": bw, "fhw": fhw, "bhw": bhw,
        "cb": cb, "hb": hb, "pad": pad,
    }
    in_maps = []
    for i in range(NCORES):
        m = dict(shared)
        # [BLOC, S, H] -> feature-major [BLOC, HC, 128, S]
        xi = x[i * BLOC:(i + 1) * BLOC].transpose(0, 2, 1)  # [BLOC, H, S]
        m["xT"] = np.ascontiguousarray(xi).reshape(BLOC, HC, 128, S)
        in_maps.append(m)
    return in_maps


def _run(inputs, trace=False, tmpdir=None):
    nc = _get_program()
    in_maps = _make_in_maps(inputs)
    res = run_bass_kernel_spmd(
        nc, in_maps, core_ids=list(range(NCORES)), trace=trace, tmpdir=tmpdir
    )
    # [L, BLOC, 8, 128, S] per core -> concat on batch -> [L, B, S, 2H] fp32
    out = np.concatenate([np.asarray(r["out"]) for r in res.results], axis=1)
    out = out.transpose(0, 1, 4, 2, 3).reshape(L, B, S, 2 * H).astype(np.float32)
    return out, res


def kernel(**inputs):
    trace = bool(int(os.environ.get("BASS_KERNEL_TRACE", "0")))
    out, _ = _run(inputs, trace=trace)
    return out


# revision 22
# speedup vs baseline: 1.0257x; 1.0257x over previous
"""Trainium2 Bass kernel for the Bengio03 Highway BiLM problem.

Math (see reference): L=3 layers, each with fwd/bwd chains. Per (layer, dir):
  padded = [front_pads(4), seq(512), back_pads(4)]          # [B, 520, H]
  pre[t] = sum_{k=0..4} padded[t + k + off] @ W[k*H:(k+1)*H]  (off=0 fwd, 4 bwd)
  x0 = relu(pre + b)
  2x highway: proj = x @ Ws[j] + bs[j]; nonlin,gate = split(proj)
              x = sigmoid(gate)*x + (1-sigmoid(gate))*relu(nonlin)
  out[l,:,:, 0:512] = f chain, [512:1024] = b chain

Implementation strategy (per core, data-parallel over batch: 4 seqs/core):
  - Everything in bf16 (fp32 PSUM accumulation): same PE rate as fp32r but
    2-byte weight loads (no LDWEIGHTS bubble), 2x DVE rate, half the DMA.
  - Activations kept feature-major in SBUF: xT tiles [128(h), 520(t)] per
    (h_chunk, b). Matmuls compute outT = W_tile.T @ xT directly (weights are
    the stationary lhsT, activations stream as rhs with N=512 tokens), so no
    transposes are needed between layers. The 5-tap conv is 5 shifted rhs
    slices accumulated in PSUM (20 matmuls of [128,128]@[128,512] per output
    chunk).
  - Input arrives pre-transposed (feature-major) from the host; output is
    stored feature-major and re-transposed on the host. The device does no
    layout work at all: only matmuls, elementwise, and linear DMAs.
  - DMA trigger instructions cost ~650ns each on the dispatching engine, so
    weights are host-packed into big blocks: 4 DMAs per conv stage, 2 per
    highway stage, 5 for all constants. Triggers round-robin sync/scalar/
    gpsimd queues (~350 GB/s aggregate).
  - Elementwise: ScalarE does relu/sigmoid (+bias, PSUM->SBUF), VectorE does
    the 3 tensor_tensor ops of the highway combine, GpSimd writes pad columns.
"""

import os
import sys

sys.path.insert(0, "/opt/trn_rl_repo")

import ml_dtypes
import numpy as np

import concourse.bass as bass
import concourse.bacc as bacc
import concourse.tile as tile
from concourse import mybir
from concourse.bass_utils import run_bass_kernel_spmd

# Problem constants (hardcoded per spec).
L = 3
WIDTH = 4
H = 512
B = 32
S = 512
NHW = 2
CIN = (WIDTH + 1) * H  # 2560
NCORES = 8
BLOC = B // NCORES  # 4 sequences per core
HC = H // 128  # 4 hidden chunks of 128
SPAD = S + 2 * WIDTH  # 520
NSEC = 4  # conv weight sections per stage (5 k-chunks each)
SECR = CIN // 128 // NSEC  # 5
F32 = mybir.dt.float32
BF16 = mybir.dt.bfloat16
RELU = mybir.ActivationFunctionType.Relu
SIGM = mybir.ActivationFunctionType.Sigmoid
ADD = mybir.AluOpType.add
MAX = mybir.AluOpType.max

NPBF16 = ml_dtypes.bfloat16


def _build_program():
    nc = bacc.Bacc(
        "TRN2",
        target_bir_lowering=False,
        debug=False,
        enable_asserts=False,
        num_devices=1,
    )

    # Input pre-transposed to feature-major on host: [b, h_chunk, h_in_chunk, t]
    xT_d = nc.dram_tensor("xT", [BLOC, HC, 128, S], BF16, kind="ExternalInput").ap()
    # Conv weights: [l, r-chunk, part, H] (per-tile DMAs)
    fw_d = nc.dram_tensor("fw", [L, CIN // 128, 128, H], BF16, kind="ExternalInput").ap()
    bw_d = nc.dram_tensor("bw", [L, CIN // 128, 128, H], BF16, kind="ExternalInput").ap()
    # Highway weights: [l, j, h_chunk, part, 2H] (per-tile DMAs)
    fhw_d = nc.dram_tensor("fhw", [L, NHW, HC, 128, 2 * H], BF16, kind="ExternalInput").ap()
    bhw_d = nc.dram_tensor("bhw", [L, NHW, HC, 128, 2 * H], BF16, kind="ExternalInput").ap()
    # Biases host-packed into single planes: [128, L*HC] / [128, L*NHW*2HC]
    cb_d = nc.dram_tensor("cb", [2, 128, L * HC], F32, kind="ExternalInput").ap()
    hb_d = nc.dram_tensor("hb", [2, 128, L * NHW * 2 * HC], F32, kind="ExternalInput").ap()
    # Pads host-packed: [2, 128, L*HC*WIDTH]
    pad_d = nc.dram_tensor("pad", [2, 128, L * HC * WIDTH], BF16, kind="ExternalInput").ap()
    # Output feature-major: [l, b, feat_chunk(2H/128=8), 128, t]; host transposes.
    out_d = nc.dram_tensor("out", [L, BLOC, 2 * HC, 128, S], BF16, kind="ExternalOutput").ap()

    with tile.TileContext(nc) as tc:
        with (
            tc.tile_pool(name="consts", bufs=1) as consts,
            tc.tile_pool(name="acts", bufs=48) as acts,
            tc.tile_pool(name="convw", bufs=40) as convw,
            tc.tile_pool(name="hww", bufs=16) as hww,
            tc.tile_pool(name="xmid", bufs=8) as xmid,
            tc.tile_pool(name="work", bufs=4) as work,
            tc.tile_pool(name="psum", bufs=2, space="PSUM") as psum,
        ):
            # ---- constants: 5 small DMAs total ----
            padt = consts.tile([128, 2, L * HC * WIDTH], BF16, name="padt", tag="padt", bufs=1)
            nc.sync.dma_start(padt[:, 0], pad_d[0])
            nc.sync.dma_start(padt[:, 1], pad_d[1])
            cbt = consts.tile([128, 2, L * HC], F32, name="cbt", tag="cbt", bufs=1)
            nc.sync.dma_start(cbt[:, 0], cb_d[0])
            nc.sync.dma_start(cbt[:, 1], cb_d[1])
            hbt = consts.tile([128, 2, L * NHW * 2 * HC], F32, name="hbt", tag="hbt", bufs=1)
            nc.sync.dma_start(hbt[:, 0], hb_d[0])
            nc.sync.dma_start(hbt[:, 1], hb_d[1])

            def cbias(dirc, l, n):
                return cbt[:, 0 if dirc == "f" else 1, l * HC + n: l * HC + n + 1]

            def hbias(dirc, l, j, c):
                o = (l * NHW + j) * 2 * HC + c
                return hbt[:, 0 if dirc == "f" else 1, o:o + 1]

            def pad_ap(dirc, l, c):
                o = (l * HC + c) * WIDTH
                return padt[:, 0 if dirc == "f" else 1, o:o + WIDTH]

            def new_act_tile(name):
                return acts.tile([128, SPAD], BF16, name=name, tag="acts", bufs=48)

            def write_pads(at, l, c):
                nc.gpsimd.tensor_copy(at[:, 0:WIDTH], pad_ap("f", l, c))
                nc.gpsimd.tensor_copy(at[:, WIDTH + S:SPAD], pad_ap("b", l, c))

            QS = (nc.sync, nc.scalar, nc.gpsimd)
            qctr = [0]

            def qnext():
                q = QS[qctr[0] % 3]
                qctr[0] += 1
                return q

            loaded_cw = {}
            loaded_hw = {}

            def ensure_convw(dirc, l):
                if (dirc, l) not in loaded_cw:
                    src = fw_d if dirc == "f" else bw_d
                    tiles = []
                    for r in range(CIN // 128):
                        w = convw.tile(
                            [128, H], BF16, name=f"cw_{dirc}{l}_{r}",
                            tag="convw", bufs=40,
                        )
                        qnext().dma_start(w[:], src[l, r])
                        tiles.append(w)
                    loaded_cw[(dirc, l)] = tiles
                return loaded_cw[(dirc, l)]

            def load_convw_halves(dirc, l):
                # Column-wave DMAs so the first conv groups (low n) can start
                # after only a fraction of the weight bytes have landed.
                src = fw_d if dirc == "f" else bw_d
                tiles = []
                for r in range(CIN // 128):
                    w = convw.tile(
                        [128, H], BF16, name=f"cw_{dirc}{l}_{r}",
                        tag="convw", bufs=40,
                    )
                    tiles.append(w)
                for c0, c1 in ((0, 128), (128, 256), (256, 512)):
                    for r in range(CIN // 128):
                        qnext().dma_start(tiles[r][:, c0:c1], src[l, r][:, c0:c1])
                loaded_cw[(dirc, l)] = tiles
                return tiles

            def ensure_hww(dirc, l):
                if (dirc, l) not in loaded_hw:
                    src = fhw_d if dirc == "f" else bhw_d
                    jt = []
                    for j in range(NHW):
                        ht = []
                        for h in range(HC):
                            w = hww.tile(
                                [128, 2 * H], BF16, name=f"hw_{dirc}{l}_{j}_{h}",
                                tag="hww", bufs=16,
                            )
                            qnext().dma_start(w[:], src[l, j, h])
                            ht.append(w)
                        jt.append(ht)
                    loaded_hw[(dirc, l)] = jt
                return loaded_hw[(dirc, l)]

            # ---- input stage: direct feature-major bf16 load ----
            # Pair (0,1) first (feeds the very first conv groups), then the
            # l0 conv weights, then pair (2,3); all round-robined over queues.
            xT = {}

            def load_x(b):
                for c in range(HC):
                    at = new_act_tile(f"xT_{c}_{b}")
                    qnext().dma_start(at[:, WIDTH:WIDTH + S], xT_d[b, c])
                    write_pads(at, 0, c)
                    xT[(c, b)] = at

            load_x(0)
            load_x(1)
            load_convw_halves("f", 0)
            load_x(2)
            load_x(3)

            # ---- stages ----
            def conv_stage(dirc, l, srcset, pair, wtiles, n_outer=False):
                off0 = 0 if dirc == "f" else WIDTH
                x0 = {}
                if n_outer:  # startup: n=0,1 groups only need the first column wave
                    groups = [(b, n) for n in range(HC) for b in pair]
                else:
                    groups = [(b, n) for b in pair for n in range(HC)]
                for b, n in groups:
                        ps = psum.tile([128, S], F32, name=f"cps_{b}_{n}", tag="cpsum", bufs=4)
                        for r in range(20):
                            k, ci = divmod(r, HC)
                            off = off0 + k
                            nc.tensor.matmul(
                                ps[:],
                                lhsT=wtiles[r][:, n * 128:(n + 1) * 128],
                                rhs=srcset[(ci, b)][:, off:off + S],
                                start=(r == 0),
                                stop=(r == 19),
                            )
                        xt = xmid.tile([128, S], BF16, name=f"x0_{b}_{n}", tag="x0", bufs=8)
                        nc.vector.tensor_scalar(
                            xt[:], ps[:], cbias(dirc, l, n), 0.0, ADD, MAX
                        )
                        x0[(n, b)] = xt
                return x0

            def hw_stage(dirc, l, j, srcset, pair, wt, final):
                outs = {}
                for b in pair:
                    for c in range(HC):
                        pnl = psum.tile([128, S], F32, name=f"hnl_{b}_{c}", tag="hpsum", bufs=4)
                        for h in range(HC):
                            nc.tensor.matmul(
                                pnl[:],
                                lhsT=wt[h][:, c * 128:(c + 1) * 128],
                                rhs=srcset[(h, b)][:],
                                start=(h == 0),
                                stop=(h == HC - 1),
                            )
                        pgt = psum.tile([128, S], F32, name=f"hgt_{b}_{c}", tag="hpsum", bufs=4)
                        for h in range(HC):
                            nc.tensor.matmul(
                                pgt[:],
                                lhsT=wt[h][:, H + c * 128:H + (c + 1) * 128],
                                rhs=srcset[(h, b)][:],
                                start=(h == 0),
                                stop=(h == HC - 1),
                            )
                        r = work.tile([128, S], BF16, name=f"r_{b}_{c}", tag="r", bufs=4)
                        nc.scalar.activation(r[:], pnl[:], RELU, bias=hbias(dirc, l, j, c))
                        g = work.tile([128, S], BF16, name=f"g_{b}_{c}", tag="g", bufs=4)
                        nc.scalar.activation(g[:], pgt[:], SIGM, bias=hbias(dirc, l, j, HC + c))
                        d = work.tile([128, S], BF16, name=f"d_{b}_{c}", tag="d", bufs=4)
                        nc.vector.tensor_sub(d[:], srcset[(c, b)][:], r[:])
                        nc.vector.tensor_mul(d[:], g[:], d[:])
                        if final:
                            at = new_act_tile(f"a_{dirc}{l}_{c}_{b}")
                            nc.vector.tensor_add(at[:, WIDTH:WIDTH + S], d[:], r[:])
                            if l + 1 < L:
                                write_pads(at, l + 1, c)
                            emit_out(dirc, l, at, c, b)
                            outs[(c, b)] = at
                        else:
                            o = xmid.tile([128, S], BF16, name=f"x1_{b}_{c}", tag="x1", bufs=8)
                            nc.vector.tensor_add(o[:], d[:], r[:])
                            outs[(c, b)] = o
                return outs

            def emit_out(dirc, l, at, c, b):
                # Feature-major store: one linear DMA per tile; the host undoes
                # the layout. Alternate dispatch queues to spread DMA load.
                # Final stage splits each store across two queues so the last
                # tiles drain faster after the final combine.
                k = (0 if dirc == "f" else HC) + c
                if dirc == "b" and l == L - 1:
                    e0 = QS[(c + b) % 3]
                    e1 = QS[(c + b + 1) % 3]
                    e0.dma_start(out_d[l, b, k][:, 0:S // 2], at[:, WIDTH:WIDTH + S // 2])
                    e1.dma_start(out_d[l, b, k][:, S // 2:], at[:, WIDTH + S // 2:WIDTH + S])
                else:
                    eng = QS[(c + b) % 3]
                    eng.dma_start(out_d[l, b, k], at[:, WIDTH:WIDTH + S])

            # ---- main chain: f fully, then b (xT stays resident for b) ----
            PAIRS = [(0, 1), (2, 3)]
            first = True
            for dirc in ("f", "b"):
                cur = xT
                for l in range(L):
                    cw = ensure_convw(dirc, l)
                    nxt = {}
                    hw = None
                    for pair in PAIRS:
                        x0 = conv_stage(dirc, l, cur, pair, cw, n_outer=first)
                        # hww DMAs emitted after the first conv groups so they
                        # don't compete with the critical startup loads.
                        if hw is None:
                            hw = ensure_hww(dirc, l)
                        x1 = hw_stage(dirc, l, 0, x0, pair, hw[0], final=False)
                        res = hw_stage(dirc, l, 1, x1, pair, hw[1], final=True)
                        nxt.update(res)
                    first = False
                    cur = nxt

    nc.compile()
    return nc


_CACHE = {}


def _get_program():
    if "nc" not in _CACHE:
        _CACHE["nc"] = _build_program()
    return _CACHE["nc"]


def _bf16(a):
    return np.ascontiguousarray(np.asarray(a, dtype=np.float32).astype(NPBF16))


def _make_in_maps(inputs):
    x = np.asarray(inputs["inputs"], dtype=np.float32).astype(NPBF16)
    # Conv weights [L, CIN, H] -> [L, 20, 128, H]
    def packw(w):
        w = np.asarray(w, dtype=np.float32).astype(NPBF16)
        return np.ascontiguousarray(w.reshape(L, CIN // 128, 128, H))

    # Highway weights [L, NHW, H, 2H] -> [L, NHW, HC, 128, 2H]
    def packhw(w):
        w = np.asarray(w, dtype=np.float32).astype(NPBF16)
        return np.ascontiguousarray(w.reshape(L, NHW, HC, 128, 2 * H))

    fw = packw(inputs["fwd_W"])
    bw = packw(inputs["bwd_W"])
    fhw = packhw(inputs["fwd_hw_W"])
    bhw = packhw(inputs["bwd_hw_W"])

    # Conv biases [L, H] -> [128, L*HC] stacked f/b
    def packcb(b):
        b = np.asarray(b, dtype=np.float32).reshape(L, HC, 128).transpose(2, 0, 1)
        return b.reshape(128, L * HC)

    cb = np.ascontiguousarray(np.stack([packcb(inputs["fwd_b"]), packcb(inputs["bwd_b"])]))

    # Highway biases [L, NHW, 2H] -> [128, L*NHW*2HC] stacked f/b
    def packhb(b):
        b = np.asarray(b, dtype=np.float32).reshape(L, NHW, 2 * HC, 128).transpose(3, 0, 1, 2)
        return b.reshape(128, L * NHW * 2 * HC)

    hb = np.ascontiguousarray(np.stack([packhb(inputs["fwd_hw_b"]), packhb(inputs["bwd_hw_b"])]))

    # Pads [L, W, H] -> [128, L*HC*W] stacked f/b
    def packpad(p):
        p = np.asarray(p, dtype=np.float32).reshape(L, WIDTH, HC, 128).transpose(3, 0, 2, 1)
        return p.reshape(128, L * HC * WIDTH).astype(NPBF16)

    pad = np.ascontiguousarray(np.stack([packpad(inputs["fwd_pads"]), packpad(inputs["bwd_pads"])]))

    shared = {
        "fw": fw, "bw": bw, "fhw": fhw, "bhw": bhw,
        "cb": cb, "hb": hb, "pad": pad,
    }
    in_maps = []
    for i in range(NCORES):
        m = dict(shared)
        # [BLOC, S, H] -> feature-major [BLOC, HC, 128, S]
        xi = x[i * BLOC:(i + 1) * BLOC].transpose(0, 2, 1)  # [BLOC, H, S]
        m["xT"] = np.ascontiguousarray(xi).reshape(BLOC, HC, 128, S)
        in_maps.append(m)
    return in_maps


def _run(inputs, trace=False, tmpdir=None):
    nc = _get_program()
    in_maps = _make_in_maps(inputs)
    res = run_bass_kernel_spmd(
        nc, in_maps, core_ids=list(range(NCORES)), trace=trace, tmpdir=tmpdir
    )
    # [L, BLOC, 8, 128, S] per core -> concat on batch -> [L, B, S, 2H] fp32
    out = np.concatenate([np.asarray(r["out"]) for r in res.results], axis=1)
    out = out.transpose(0, 1, 4, 2, 3).reshape(L, B, S, 2 * H).astype(np.float32)
    return out, res


def kernel(**inputs):
    trace = bool(int(os.environ.get("BASS_KERNEL_TRACE", "0")))
    out, _ = _run(inputs, trace=trace)
    return out


# revision 23
# speedup vs baseline: 1.0321x; 1.0062x over previous
"""Trainium2 Bass kernel for the Bengio03 Highway BiLM problem.

Math (see reference): L=3 layers, each with fwd/bwd chains. Per (layer, dir):
  padded = [front_pads(4), seq(512), back_pads(4)]          # [B, 520, H]
  pre[t] = sum_{k=0..4} padded[t + k + off] @ W[k*H:(k+1)*H]  (off=0 fwd, 4 bwd)
  x0 = relu(pre + b)
  2x highway: proj = x @ Ws[j] + bs[j]; nonlin,gate = split(proj)
              x = sigmoid(gate)*x + (1-sigmoid(gate))*relu(nonlin)
  out[l,:,:, 0:512] = f chain, [512:1024] = b chain

Implementation strategy (per core, data-parallel over batch: 4 seqs/core):
  - Everything in bf16 (fp32 PSUM accumulation): same PE rate as fp32r but
    2-byte weight loads (no LDWEIGHTS bubble), 2x DVE rate, half the DMA.
  - Activations kept feature-major in SBUF: xT tiles [128(h), 520(t)] per
    (h_chunk, b). Matmuls compute outT = W_tile.T @ xT directly (weights are
    the stationary lhsT, activations stream as rhs with N=512 tokens), so no
    transposes are needed between layers. The 5-tap conv is 5 shifted rhs
    slices accumulated in PSUM (20 matmuls of [128,128]@[128,512] per output
    chunk).
  - Input arrives pre-transposed (feature-major) from the host; output is
    stored feature-major and re-transposed on the host. The device does no
    layout work at all: only matmuls, elementwise, and linear DMAs.
  - DMA trigger instructions cost ~650ns each on the dispatching engine and
    all constants load in 5 merged DMAs; weights stay as per-tile <=256KB
    DMAs round-robined over the sync/scalar/gpsimd queues (~350 GB/s
    aggregate burst). Layer-0 conv weights stream in three column waves and
    the first conv pair iterates n-outer so the PE starts on a quarter of
    the critical bytes; highway weights load after the first conv groups.
  - Elementwise: ScalarE does relu/sigmoid (+bias, PSUM->SBUF), VectorE does
    the 3 tensor_tensor ops of the highway combine, GpSimd writes pad columns.
"""

import os
import sys

sys.path.insert(0, "/opt/trn_rl_repo")

import ml_dtypes
import numpy as np

import concourse.bass as bass
import concourse.bacc as bacc
import concourse.tile as tile
from concourse import mybir
from concourse.bass_utils import run_bass_kernel_spmd

# Problem constants (hardcoded per spec).
L = 3
WIDTH = 4
H = 512
B = 32
S = 512
NHW = 2
CIN = (WIDTH + 1) * H  # 2560
NCORES = 8
BLOC = B // NCORES  # 4 sequences per core
HC = H // 128  # 4 hidden chunks of 128
SPAD = S + 2 * WIDTH  # 520
NSEC = 4  # conv weight sections per stage (5 k-chunks each)
SECR = CIN // 128 // NSEC  # 5
F32 = mybir.dt.float32
BF16 = mybir.dt.bfloat16
RELU = mybir.ActivationFunctionType.Relu
SIGM = mybir.ActivationFunctionType.Sigmoid
ADD = mybir.AluOpType.add
MAX = mybir.AluOpType.max

NPBF16 = ml_dtypes.bfloat16


def _build_program():
    nc = bacc.Bacc(
        "TRN2",
        target_bir_lowering=False,
        debug=False,
        enable_asserts=False,
        num_devices=1,
    )

    # Input pre-transposed to feature-major on host: [b, h_chunk, h_in_chunk, t]
    xT_d = nc.dram_tensor("xT", [BLOC, HC, 128, S], BF16, kind="ExternalInput").ap()
    # Conv weights: [l, r-chunk, part, H] (per-tile DMAs)
    fw_d = nc.dram_tensor("fw", [L, CIN // 128, 128, H], BF16, kind="ExternalInput").ap()
    bw_d = nc.dram_tensor("bw", [L, CIN // 128, 128, H], BF16, kind="ExternalInput").ap()
    # Highway weights: [l, j, h_chunk, part, 2H] (per-tile DMAs)
    fhw_d = nc.dram_tensor("fhw", [L, NHW, HC, 128, 2 * H], BF16, kind="ExternalInput").ap()
    bhw_d = nc.dram_tensor("bhw", [L, NHW, HC, 128, 2 * H], BF16, kind="ExternalInput").ap()
    # Biases host-packed into single planes: [128, L*HC] / [128, L*NHW*2HC]
    cb_d = nc.dram_tensor("cb", [2, 128, L * HC], F32, kind="ExternalInput").ap()
    hb_d = nc.dram_tensor("hb", [2, 128, L * NHW * 2 * HC], F32, kind="ExternalInput").ap()
    # Pads host-packed: [2, 128, L*HC*WIDTH]
    pad_d = nc.dram_tensor("pad", [2, 128, L * HC * WIDTH], BF16, kind="ExternalInput").ap()
    # Output feature-major: [l, b, feat_chunk(2H/128=8), 128, t]; host transposes.
    out_d = nc.dram_tensor("out", [L, BLOC, 2 * HC, 128, S], BF16, kind="ExternalOutput").ap()

    with tile.TileContext(nc) as tc:
        with (
            tc.tile_pool(name="consts", bufs=1) as consts,
            tc.tile_pool(name="acts", bufs=48) as acts,
            tc.tile_pool(name="convw", bufs=40) as convw,
            tc.tile_pool(name="hww", bufs=16) as hww,
            tc.tile_pool(name="xmid", bufs=8) as xmid,
            tc.tile_pool(name="work", bufs=4) as work,
            tc.tile_pool(name="psum", bufs=2, space="PSUM") as psum,
        ):
            # ---- constants: 5 small DMAs total ----
            padt = consts.tile([128, 2, L * HC * WIDTH], BF16, name="padt", tag="padt", bufs=1)
            nc.sync.dma_start(padt[:, 0], pad_d[0])
            nc.sync.dma_start(padt[:, 1], pad_d[1])
            cbt = consts.tile([128, 2, L * HC], F32, name="cbt", tag="cbt", bufs=1)
            nc.sync.dma_start(cbt[:, 0], cb_d[0])
            nc.sync.dma_start(cbt[:, 1], cb_d[1])
            hbt = consts.tile([128, 2, L * NHW * 2 * HC], F32, name="hbt", tag="hbt", bufs=1)
            nc.sync.dma_start(hbt[:, 0], hb_d[0])
            nc.sync.dma_start(hbt[:, 1], hb_d[1])

            def cbias(dirc, l, n):
                return cbt[:, 0 if dirc == "f" else 1, l * HC + n: l * HC + n + 1]

            def hbias(dirc, l, j, c):
                o = (l * NHW + j) * 2 * HC + c
                return hbt[:, 0 if dirc == "f" else 1, o:o + 1]

            def pad_ap(dirc, l, c):
                o = (l * HC + c) * WIDTH
                return padt[:, 0 if dirc == "f" else 1, o:o + WIDTH]

            def new_act_tile(name):
                return acts.tile([128, SPAD], BF16, name=name, tag="acts", bufs=48)

            def write_pads(at, l, c):
                nc.gpsimd.tensor_copy(at[:, 0:WIDTH], pad_ap("f", l, c))
                nc.gpsimd.tensor_copy(at[:, WIDTH + S:SPAD], pad_ap("b", l, c))

            QS = (nc.sync, nc.scalar, nc.gpsimd)
            qctr = [0]

            def qnext():
                q = QS[qctr[0] % 3]
                qctr[0] += 1
                return q

            loaded_cw = {}
            loaded_hw = {}

            def ensure_convw(dirc, l):
                if (dirc, l) not in loaded_cw:
                    src = fw_d if dirc == "f" else bw_d
                    tiles = []
                    for r in range(CIN // 128):
                        w = convw.tile(
                            [128, H], BF16, name=f"cw_{dirc}{l}_{r}",
                            tag="convw", bufs=40,
                        )
                        qnext().dma_start(w[:], src[l, r])
                        tiles.append(w)
                    loaded_cw[(dirc, l)] = tiles
                return loaded_cw[(dirc, l)]

            def load_convw_halves(dirc, l):
                # Column-wave DMAs so the first conv groups (low n) can start
                # after only a fraction of the weight bytes have landed.
                src = fw_d if dirc == "f" else bw_d
                tiles = []
                for r in range(CIN // 128):
                    w = convw.tile(
                        [128, H], BF16, name=f"cw_{dirc}{l}_{r}",
                        tag="convw", bufs=40,
                    )
                    tiles.append(w)
                for c0, c1 in ((0, 128), (128, 256), (256, 512)):
                    for r in range(CIN // 128):
                        qnext().dma_start(tiles[r][:, c0:c1], src[l, r][:, c0:c1])
                loaded_cw[(dirc, l)] = tiles
                return tiles

            def ensure_hww(dirc, l):
                if (dirc, l) not in loaded_hw:
                    src = fhw_d if dirc == "f" else bhw_d
                    jt = []
                    for j in range(NHW):
                        ht = []
                        for h in range(HC):
                            w = hww.tile(
                                [128, 2 * H], BF16, name=f"hw_{dirc}{l}_{j}_{h}",
                                tag="hww", bufs=16,
                            )
                            qnext().dma_start(w[:], src[l, j, h])
                            ht.append(w)
                        jt.append(ht)
                    loaded_hw[(dirc, l)] = jt
                return loaded_hw[(dirc, l)]

            # ---- input stage: direct feature-major bf16 load ----
            # Pair (0,1) first (feeds the very first conv groups), then the
            # l0 conv weights, then pair (2,3); all round-robined over queues.
            xT = {}

            def load_x(b):
                for c in range(HC):
                    at = new_act_tile(f"xT_{c}_{b}")
                    qnext().dma_start(at[:, WIDTH:WIDTH + S], xT_d[b, c])
                    write_pads(at, 0, c)
                    xT[(c, b)] = at

            load_x(0)
            load_x(1)
            load_convw_halves("f", 0)
            load_x(2)
            load_x(3)

            # ---- stages ----
            def conv_stage(dirc, l, srcset, pair, wtiles, n_outer=False):
                off0 = 0 if dirc == "f" else WIDTH
                x0 = {}
                if n_outer:  # startup: n=0,1 groups only need the first column wave
                    groups = [(b, n) for n in range(HC) for b in pair]
                else:
                    groups = [(b, n) for b in pair for n in range(HC)]
                for b, n in groups:
                        ps = psum.tile([128, S], F32, name=f"cps_{b}_{n}", tag="cpsum", bufs=4)
                        for r in range(20):
                            k, ci = divmod(r, HC)
                            off = off0 + k
                            nc.tensor.matmul(
                                ps[:],
                                lhsT=wtiles[r][:, n * 128:(n + 1) * 128],
                                rhs=srcset[(ci, b)][:, off:off + S],
                                start=(r == 0),
                                stop=(r == 19),
                            )
                        xt = xmid.tile([128, S], BF16, name=f"x0_{b}_{n}", tag="x0", bufs=8)
                        nc.vector.tensor_scalar(
                            xt[:], ps[:], cbias(dirc, l, n), 0.0, ADD, MAX
                        )
                        x0[(n, b)] = xt
                return x0

            def hw_stage(dirc, l, j, srcset, pair, wt, final):
                outs = {}
                for b in pair:
                    for c in range(HC):
                        pnl = psum.tile([128, S], F32, name=f"hnl_{b}_{c}", tag="hpsum", bufs=4)
                        for h in range(HC):
                            nc.tensor.matmul(
                                pnl[:],
                                lhsT=wt[h][:, c * 128:(c + 1) * 128],
                                rhs=srcset[(h, b)][:],
                                start=(h == 0),
                                stop=(h == HC - 1),
                            )
                        pgt = psum.tile([128, S], F32, name=f"hgt_{b}_{c}", tag="hpsum", bufs=4)
                        for h in range(HC):
                            nc.tensor.matmul(
                                pgt[:],
                                lhsT=wt[h][:, H + c * 128:H + (c + 1) * 128],
                                rhs=srcset[(h, b)][:],
                                start=(h == 0),
                                stop=(h == HC - 1),
                            )
                        r = work.tile([128, S], BF16, name=f"r_{b}_{c}", tag="r", bufs=4)
                        nc.scalar.activation(r[:], pnl[:], RELU, bias=hbias(dirc, l, j, c))
                        g = work.tile([128, S], BF16, name=f"g_{b}_{c}", tag="g", bufs=4)
                        nc.scalar.activation(g[:], pgt[:], SIGM, bias=hbias(dirc, l, j, HC + c))
                        d = work.tile([128, S], BF16, name=f"d_{b}_{c}", tag="d", bufs=4)
                        nc.vector.tensor_sub(d[:], srcset[(c, b)][:], r[:])
                        nc.vector.tensor_mul(d[:], g[:], d[:])
                        if final:
                            at = new_act_tile(f"a_{dirc}{l}_{c}_{b}")
                            nc.vector.tensor_add(at[:, WIDTH:WIDTH + S], d[:], r[:])
                            if l + 1 < L:
                                write_pads(at, l + 1, c)
                            emit_out(dirc, l, at, c, b)
                            outs[(c, b)] = at
                        else:
                            o = xmid.tile([128, S], BF16, name=f"x1_{b}_{c}", tag="x1", bufs=8)
                            nc.vector.tensor_add(o[:], d[:], r[:])
                            outs[(c, b)] = o
                return outs

            def emit_out(dirc, l, at, c, b):
                # Feature-major store: one linear DMA per tile; the host undoes
                # the layout. Alternate dispatch queues to spread DMA load.
                # Final stage splits each store across two queues so the last
                # tiles drain faster after the final combine.
                k = (0 if dirc == "f" else HC) + c
                if dirc == "b" and l == L - 1:
                    e0 = QS[(c + b) % 3]
                    e1 = QS[(c + b + 1) % 3]
                    e0.dma_start(out_d[l, b, k][:, 0:S // 2], at[:, WIDTH:WIDTH + S // 2])
                    e1.dma_start(out_d[l, b, k][:, S // 2:], at[:, WIDTH + S // 2:WIDTH + S])
                else:
                    eng = QS[(c + b) % 3]
                    eng.dma_start(out_d[l, b, k], at[:, WIDTH:WIDTH + S])

            # ---- main chain: f fully, then b (xT stays resident for b) ----
            PAIRS = [(0, 1), (2, 3)]
            first = True
            for dirc in ("f", "b"):
                cur = xT
                for l in range(L):
                    cw = ensure_convw(dirc, l)
                    nxt = {}
                    hw = None
                    for pair in PAIRS:
                        x0 = conv_stage(dirc, l, cur, pair, cw, n_outer=first)
                        # hww DMAs emitted after the first conv groups so they
                        # don't compete with the critical startup loads.
                        if hw is None:
                            hw = ensure_hww(dirc, l)
                        x1 = hw_stage(dirc, l, 0, x0, pair, hw[0], final=False)
                        res = hw_stage(dirc, l, 1, x1, pair, hw[1], final=True)
                        nxt.update(res)
                    first = False
                    cur = nxt

    nc.compile()
    return nc


_CACHE = {}


def _get_program():
    if "nc" not in _CACHE:
        _CACHE["nc"] = _build_program()
    return _CACHE["nc"]


def _bf16(a):
    return np.ascontiguousarray(np.asarray(a, dtype=np.float32).astype(NPBF16))


def _make_in_maps(inputs):
    x = np.asarray(inputs["inputs"], dtype=np.float32).astype(NPBF16)
    # Conv weights [L, CIN, H] -> [L, 20, 128, H]
    def packw(w):
        w = np.asarray(w, dtype=np.float32).astype(NPBF16)
        return np.ascontiguousarray(w.reshape(L, CIN // 128, 128, H))

    # Highway weights [L, NHW, H, 2H] -> [L, NHW, HC, 128, 2H]
    def packhw(w):
        w = np.asarray(w, dtype=np.float32).astype(NPBF16)
        return np.ascontiguousarray(w.reshape(L, NHW, HC, 128, 2 * H))

    fw = packw(inputs["fwd_W"])
    bw = packw(inputs["bwd_W"])
    fhw = packhw(inputs["fwd_hw_W"])
    bhw = packhw(inputs["bwd_hw_W"])

    # Conv biases [L, H] -> [128, L*HC] stacked f/b
    def packcb(b):
        b = np.asarray(b, dtype=np.float32).reshape(L, HC, 128).transpose(2, 0, 1)
        return b.reshape(128, L * HC)

    cb = np.ascontiguousarray(np.stack([packcb(inputs["fwd_b"]), packcb(inputs["bwd_b"])]))

    # Highway biases [L, NHW, 2H] -> [128, L*NHW*2HC] stacked f/b
    def packhb(b):
        b = np.asarray(b, dtype=np.float32).reshape(L, NHW, 2 * HC, 128).transpose(3, 0, 1, 2)
        return b.reshape(128, L * NHW * 2 * HC)

    hb = np.ascontiguousarray(np.stack([packhb(inputs["fwd_hw_b"]), packhb(inputs["bwd_hw_b"])]))

    # Pads [L, W, H] -> [128, L*HC*W] stacked f/b
    def packpad(p):
        p = np.asarray(p, dtype=np.float32).reshape(L, WIDTH, HC, 128).transpose(3, 0, 2, 1)
        return p.reshape(128, L * HC * WIDTH).astype(NPBF16)

    pad = np.ascontiguousarray(np.stack([packpad(inputs["fwd_pads"]), packpad(inputs["bwd_pads"])]))

    shared = {
        "fw": fw, "bw": bw, "fhw": fhw, "bhw": bhw,
        "cb": cb, "hb": hb, "pad": pad,
    }
    in_maps = []
    for i in range(NCORES):
        m = dict(shared)
        # [BLOC, S, H] -> feature-major [BLOC, HC, 128, S]
        xi = x[i * BLOC:(i + 1) * BLOC].transpose(0, 2, 1)  # [BLOC, H, S]
        m["xT"] = np.ascontiguousarray(xi).reshape(BLOC, HC, 128, S)
        in_maps.append(m)
    return in_maps


def _run(inputs, trace=False, tmpdir=None):
    nc = _get_program()
    in_maps = _make_in_maps(inputs)
    res = run_bass_kernel_spmd(
        nc, in_maps, core_ids=list(range(NCORES)), trace=trace, tmpdir=tmpdir
    )
    # [L, BLOC, 8, 128, S] per core -> concat on batch -> [L, B, S, 2H] fp32
    out = np.concatenate([np.asarray(r["out"]) for r in res.results], axis=1)
    out = out.transpose(0, 1, 4, 2, 3).reshape(L, B, S, 2 * H).astype(np.float32)
    return out, res


def kernel(**inputs):
    trace = bool(int(os.environ.get("BASS_KERNEL_TRACE", "0")))
    out, _ = _run(inputs, trace=trace)
    return out
